# revision 34
# baseline (speedup 1.0000x reference)
"""AceStep GQA attention block on 8 Trainium2 NeuronCores.

Sharding: DP over batch (B=2) x sequence-parallel over S within each batch
group (4 cores each own 512 query positions).  Each core computes K/V for its
own 512 positions, AllGathers K^T/V (bf16) within its 4-core group, then runs
full attention for its query slice and the complete output projection row
block.  No output collective needed - each core owns a distinct slice.

v7 schedule (vs v2 at ~409us measured): the PE issues N=512 bf16 matmuls at a
steady ~263ns (power-throttle ceiling); all remaining headroom is PE idle
gaps, so v7 is a pure scheduling restructure:
  - input DMAs chunked and split across the two HWDGE queues (sync: weights
    + packs/unpacks, scalar: hidden/rope consts); w_pool is 3-deep so each
    wq chunk prefetches a full block ahead.  gpsimd carries ONLY the
    collective triggers (software DMAs there contend with the CC DSP).
  - K projection runs FIRST: measured, the first collective cannot finish
    before ~90us regardless of issue time (ring init + cross-core skew), so
    the K gather -- whose payload attention needs first -- is the one that
    absorbs that floor.  V follows (its gather rides second; attention's
    AV matmuls lag scores anyway, and pair 0 uses a deeper AV lag to ride
    out the later v arrival).  Q projection overlaps both transfers.
  - per-tile norm scales (Copy+reciprocal+Sqrt -- one ACT table set with
    Square, no Ln/Exp thrash) + an ACT psum->SBUF stage right after each
    chain's squares: psum banks recycle ~0.7us after their chain and the
    DVE ropes pipeline tt-by-tt under the matmuls.
  - K/Q rope->transpose tails run on the PE but are DEFERRED two tt-chains
    past their rope's ready time (in-order PE queue): each 4-transpose
    group is emitted at a later chain boundary where its DVE rope is
    provably done, so the PE never stalls.  The last 5 groups (oc2-tt3,
    oc3) go through the DMA XBAR on sync -- needed only by attention pairs
    4-7, ~90us after they land.
  - softmax denominator tree-adds interleave into the score loop (p2 is a
    10-slot rotating buffer); each pair's den-matmul/reciprocal/attnT tail
    is emitted inside the NEXT pair's loop so the PE never waits on DVE.
  - O projection accumulates fl-pairs in the score psum slots ([128,1024]
    double-buffered, freed by then) and starts during the last pair's
    softmax tail; output DMAs are 8x512KB contiguous on sync

Device layouts (per core, c = 4*b + j):
  hsP   [128, it 16, t 512]  hidden[b].T tile-major (partition = i % 128)
  wkP/wvP [128, it 16, n 512]   W^T tile-major
  wqP/woP [oc 4][128, it/kk 16, n 512]
  cw*/sw* [128, tt 4, d 128]  rope coeffs (norm weight + scale folded in)
  outT  [HS, SC] f32
"""

import numpy as np

H, KV, D = 16, 4, 128
HD = D // 2
B, S, HS = 2, 2048, 2048
EPS = 1e-6
NCORES = 8
TPG = 4              # cores per batch group (sequence split)
SC = S // TPG        # 512 sequence positions per core
TT = SC // 128       # 4 t-tiles per core
IT = HS // 128       # 16 contraction tiles
ST = S // 128        # 16 s-tiles (full sequence)
OC = 4               # 512-wide output chunks for Q/O projections
GROUPS = [[0, 1, 2, 3], [4, 5, 6, 7]]

_BUILT = {}


def _build_program():
    from contextlib import ExitStack

    import concourse.bass as bass
    import concourse.bacc as bacc
    import concourse.mybir as mybir
    import concourse.tile as tile
    from concourse.masks import make_identity

    f32 = mybir.dt.float32
    bf16 = mybir.dt.bfloat16
    AF = mybir.ActivationFunctionType
    ALU = mybir.AluOpType

    nc = bacc.Bacc("TRN2", target_bir_lowering=False, debug=False,
                   num_devices=NCORES)

    # ---- external I/O (per core) ----
    hsP = nc.dram_tensor("hsP", [128, IT * SC], bf16, kind="ExternalInput").ap()
    wkP = nc.dram_tensor("wkP", [128, IT * 512], bf16, kind="ExternalInput").ap()
    wvP = nc.dram_tensor("wvP", [128, IT * 512], bf16, kind="ExternalInput").ap()
    wqP = nc.dram_tensor("wqP", [OC, 128, IT * 512], bf16,
                         kind="ExternalInput").ap()
    woP = nc.dram_tensor("woP", [OC, 128, IT * 512], bf16,
                         kind="ExternalInput").ap()
    cwq = nc.dram_tensor("cwq", [128, TT * D], f32, kind="ExternalInput").ap()
    swq = nc.dram_tensor("swq", [128, TT * D], f32, kind="ExternalInput").ap()
    cwk = nc.dram_tensor("cwk", [128, TT * D], f32, kind="ExternalInput").ap()
    swk = nc.dram_tensor("swk", [128, TT * D], f32, kind="ExternalInput").ap()
    outT = nc.dram_tensor("outT", [HS, SC], f32, kind="ExternalOutput").ap()

    tc_cm = tile.TileContext(nc)
    ctx = ExitStack()
    tc = tc_cm.__enter__()
    try:
        ep = ctx.enter_context
        const_pool = ep(tc.tile_pool(name="const", bufs=1))
        w_pool = ep(tc.tile_pool(name="w", bufs=3))
        scr_pool = ep(tc.tile_pool(name="scr", bufs=2))
        rope_pool = ep(tc.tile_pool(name="rope", bufs=4))
        qT_pool = ep(tc.tile_pool(name="qT", bufs=1))
        kv_pool = ep(tc.tile_pool(name="kv", bufs=1))
        osb_pool = ep(tc.tile_pool(name="osb", bufs=2))
        dram_pool = ep(tc.tile_pool(name="dram", bufs=1, space="DRAM"))

        # scoped pools released before the attention phase needs the space
        # (pool releases must be LIFO: kvloc closes first, then hs)
        ctx_hs = ExitStack()      # hs + projection psum (until Q proj done)
        ctx_kv = ExitStack()      # local kT/v staging (until pack DMAs done)
        hs_pool = ctx_hs.enter_context(tc.tile_pool(name="hs", bufs=1))
        mm_ps = ctx_hs.enter_context(
            tc.tile_pool(name="mm_ps", bufs=6, space="PSUM"))
        tr_ps = ctx_hs.enter_context(
            tc.tile_pool(name="tr_ps", bufs=2, space="PSUM"))
        kvloc_pool = ctx_kv.enter_context(tc.tile_pool(name="kvloc", bufs=1))

        # ---- constants ----
        ident = const_pool.tile([128, 128], bf16)
        make_identity(nc, ident)
        ones_bf = const_pool.tile([128, 128], bf16)
        nc.vector.memset(ones_bf, 1.0)
        eps_sb = const_pool.tile([128, 1], f32)
        nc.vector.memset(eps_sb, EPS)

        # hidden states: 4 chunks on the scalar (ACT) HWDGE queue so the
        # first matmul can start ~11us in; weights stream on sync
        hs_sb = hs_pool.tile([128, IT * SC], bf16)
        # first chunk small so the first K chain starts ASAP
        hcuts = [0, 512, 2048, 4096, 6144, IT * SC]
        for a, b in zip(hcuts, hcuts[1:]):
            nc.scalar.dma_start(out=hs_sb[:, a:b], in_=hsP[:, a:b])

        cwq_sb = const_pool.tile([128, TT * D], f32)
        swq_sb = const_pool.tile([128, TT * D], f32)
        cwk_sb = const_pool.tile([128, TT * D], f32)
        swk_sb = const_pool.tile([128, TT * D], f32)
        for (dst, src) in ((cwk_sb, cwk), (swk_sb, swk),
                           (cwq_sb, cwq), (swq_sb, swq)):
            nc.scalar.dma_start(out=dst[:], in_=src)

        def hs_tile(it, tt):
            # stationary [128 i, 128 t]
            off = it * SC + tt * 128
            return hs_sb[:, off: off + 128]

        def squares(ps, nh, ssum, base):
            """sum(x^2) over D per head via ACT Square + accum_out."""
            for hh in range(nh):
                sqd = scr_pool.tile([128, D], f32, tag="sqd", name="sqd")
                nc.scalar.activation(
                    sqd[:], ps[:, hh * D:(hh + 1) * D], AF.Square,
                    accum_out=ssum[:, base + hh:base + hh + 1])

        def scales_of(ssum_sl, n):
            """rsqrt(x/D + eps) = sqrt(1/(x/D + eps)): Copy + Sqrt share one
            ACT table set with Square -> the projections never swap tables
            (Exp loads once for attention)."""
            mn = scr_pool.tile([128, n], f32, tag="lnm", name="mn")
            nc.scalar.activation(mn[:], ssum_sl, AF.Copy, bias=EPS,
                                 scale=1.0 / D)
            rp = scr_pool.tile([128, n], f32, tag="rp", name="rp")
            nc.vector.reciprocal(rp[:], mn[:])
            sc_t = scr_pool.tile([128, n], f32, tag="sct", name="sc_t")
            nc.scalar.activation(sc_t[:], rp[:], AF.Sqrt)
            return sc_t

        def rope_apply(xsb, nh, sc_sl, cw_t, sw_t, dst):
            """xsb: SBUF bf16 [128 t, nh*D]; sc_sl: [128, nh] scales;
            dst: bf16 [128, nh*D].  All heads batched per DVE op."""
            scf = scr_pool.tile([128, nh * D], bf16, tag="scf", name="scf")
            nc.vector.tensor_copy(
                scf.rearrange("p (h d) -> p h d", d=D),
                sc_sl.rearrange("p (h one) -> p h one", one=1).broadcast_to(
                    [128, nh, D]))
            xs = scr_pool.tile([128, nh * D], bf16, tag="xs", name="xs")
            nc.vector.tensor_mul(xs[:], xsb[:], scf[:])
            t1 = scr_pool.tile([128, nh * D], bf16, tag="t1", name="t1")
            cwb = cw_t.rearrange("p (one d) -> p one d", one=1).broadcast_to(
                [128, nh, D])
            nc.vector.tensor_mul(t1.rearrange("p (h d) -> p h d", d=D),
                                 xs.rearrange("p (h d) -> p h d", d=D), cwb)
            t2 = scr_pool.tile([128, nh * D], bf16, tag="t2", name="t2")
            xsv = xs.rearrange("p (h two x) -> p h two x", two=2, x=HD)
            t2v = t2.rearrange("p (h two x) -> p h two x", two=2, x=HD)
            swb_lo = sw_t[:, 0:HD].rearrange(
                "p (one x) -> p one x", one=1).broadcast_to([128, nh, HD])
            swb_hi = sw_t[:, HD:D].rearrange(
                "p (one x) -> p one x", one=1).broadcast_to([128, nh, HD])
            nc.vector.tensor_mul(t2v[:, :, 0, :], xsv[:, :, 1, :], swb_lo)
            nc.vector.tensor_mul(t2v[:, :, 1, :], xsv[:, :, 0, :], swb_hi)
            nc.vector.tensor_add(dst[:], t1[:], t2[:])

        # deferred PE-transpose groups, popped one per tt-chain boundary
        # (shift-2: two leading Nones) so the in-order PE queue never
        # stalls on a DVE rope.
        pending_trs = [None, None]

        def boundary():
            if pending_trs:
                emit = pending_trs.pop(0)
                if emit is not None:
                    emit()

        # ================= K projection (first) =================
        wk_sb = w_pool.tile([128, IT * 512], bf16, tag="w", name="wk_sb")
        wcuts = [0, 512, 2048, 4096, 6144, IT * 512]
        for a, b in zip(wcuts, wcuts[1:]):
            nc.sync.dma_start(out=wk_sb[:, a:b], in_=wkP[:, a:b])
        wv_sb = w_pool.tile([128, IT * 512], bf16, tag="w", name="wv_sb")
        NWC = 4
        wchunk = IT * 512 // NWC
        for c in range(NWC):
            nc.sync.dma_start(out=wv_sb[:, c * wchunk:(c + 1) * wchunk],
                              in_=wvP[:, c * wchunk:(c + 1) * wchunk])
        wq_sb0 = w_pool.tile([128, IT * 512], bf16, tag="w", name="wq_sb")
        nc.sync.dma_start(out=wq_sb0[:], in_=wqP[0])

        kTs_sb = kvloc_pool.tile([128, KV * SC], bf16)  # [d, (g s)] local kT
        v_sb = kvloc_pool.tile([128, TT * KV * D], bf16)  # [s,(st hd)] local
        kTs_v = kTs_sb.rearrange("p (g s) -> p g s", s=SC)

        def k_tail(st, krope):
            def emit():
                pst = tr_ps.tile([128, KV * 128], bf16, tag="tr", name="pst")
                for g in range(KV):
                    nc.tensor.transpose(pst[:, g * 128:(g + 1) * 128],
                                        krope[:, g * D:(g + 1) * D],
                                        ident[:])
                nc.vector.tensor_copy(
                    kTs_v[:, :, st * 128:(st + 1) * 128],
                    pst.rearrange("p (g t) -> p g t", t=128))
            return emit

        ssum_k = scr_pool.tile([128, TT * KV], f32, tag="ssum", name="ssum_k")
        for st in range(TT):
            ps_k = mm_ps.tile([128, KV * D], f32, tag="mm", name="ps_k")
            for it in range(IT):
                nc.tensor.matmul(ps_k[:], hs_tile(it, st),
                                 wk_sb[:, it * 512:(it + 1) * 512],
                                 start=(it == 0), stop=(it == IT - 1))
            squares(ps_k, KV, ssum_k, st * KV)
            kst = scr_pool.tile([128, KV * D], bf16, tag="qps", name="kst",
                                bufs=4)
            nc.scalar.activation(kst[:], ps_k[:], AF.Copy)
            sck = scales_of(ssum_k[:, st * KV:(st + 1) * KV], KV)
            krope = rope_pool.tile([128, KV * D], bf16, tag="krope",
                                   name="krope")
            rope_apply(kst, KV, sck,
                       cwk_sb[:, st * D:(st + 1) * D],
                       swk_sb[:, st * D:(st + 1) * D], krope)
            pending_trs.append(k_tail(st, krope))

        # ================= V projection =================
        for st in range(TT):
            ps_v = mm_ps.tile([128, KV * D], f32, tag="mm", name="ps_v")
            for it in range(IT):
                nc.tensor.matmul(ps_v[:], hs_tile(it, st),
                                 wv_sb[:, it * 512:(it + 1) * 512],
                                 start=(it == 0), stop=(it == IT - 1))
            nc.vector.tensor_copy(v_sb[:, st * KV * D:(st + 1) * KV * D],
                                  ps_v[:])
            boundary()

        # v pack rides sync early; the CC triggers (gpsimd) are emitted
        # k-first at the oc0-tt1 boundary below.  Payloads are
        # partition-major [128, 2048] so every pack/unpack moves 4KB rows
        # (full-bandwidth descriptors, no rearrange).
        cc_in_k = dram_pool.tile([128, KV * SC], bf16)
        cc_out_k = dram_pool.tile([TPG * 128, KV * SC], bf16)
        cc_in_v = dram_pool.tile([128, TT * KV * D], bf16)
        cc_out_v = dram_pool.tile([TPG * 128, TT * KV * D], bf16)
        nc.sync.dma_start(out=cc_in_v[:], in_=v_sb[:])

        def emit_gathers():
            nc.sync.dma_start(out=cc_in_k[:], in_=kTs_sb[:])
            nc.gpsimd.collective_compute(
                "AllGather", ALU.bypass, replica_groups=GROUPS,
                ins=[cc_in_k.opt()], outs=[cc_out_k.opt()])
            nc.gpsimd.collective_compute(
                "AllGather", ALU.bypass, replica_groups=GROUPS,
                ins=[cc_in_v.opt()], outs=[cc_out_v.opt()])

        # gathered K/V landing buffers; unpacks emitted inside the Q loop
        kT_big = kv_pool.tile([128, KV * S], bf16, tag="kT")    # [d,(rr g s)]
        v_big = kv_pool.tile([128, ST * KV * D], bf16, tag="v")  # [s,(st hd)]

        def emit_unpack_v():
            for rr in range(TPG):
                nc.sync.dma_start(
                    out=v_big[:, rr * TT * KV * D:(rr + 1) * TT * KV * D],
                    in_=cc_out_v[rr * 128:(rr + 1) * 128, :])

        def emit_unpack_k():
            for rr in range(TPG):
                nc.sync.dma_start(
                    out=kT_big[:, rr * KV * SC:(rr + 1) * KV * SC],
                    in_=cc_out_k[rr * 128:(rr + 1) * 128, :])

        def kT_tile(g, st):
            # stationary [128 d, 128 s] for s-tile st (st = rr*4 + sub)
            rr, sub = st // TT, st % TT
            off = (rr * KV + g) * SC + sub * 128
            return kT_big[:, off: off + 128]

        def v_tile(g, st):
            # stationary [128 s, 128 d] for s-tile st
            off = st * KV * D + g * D
            return v_big[:, off: off + D]

        # ================= Q projection (overlaps collectives) ===========
        qT_sb = qT_pool.tile([128, H * SC], bf16)   # per head: [d, 512 t]
        qT_v = qT_sb.rearrange("p (h t) -> p h t", t=SC)

        def q_tail(oc, tt, qrope):
            def emit():
                pstq = tr_ps.tile([128, 4 * 128], bf16, tag="tr", name="pstq")
                for hh in range(4):
                    nc.tensor.transpose(pstq[:, hh * 128:(hh + 1) * 128],
                                        qrope[:, hh * D:(hh + 1) * D],
                                        ident[:])
                nc.vector.tensor_copy(
                    qT_v[:, oc * 4:(oc + 1) * 4, tt * 128:(tt + 1) * 128],
                    pstq.rearrange("p (h t) -> p h t", t=128))
            return emit

        xbar_trs = []   # (oc, tt, qrope) for the last 5 groups -> DMA XBAR
        wq_tiles = [wq_sb0]

        for oc in range(OC):
            if oc + 1 < OC:
                # prefetch the next oc's weights one block ahead
                nxt = w_pool.tile([128, IT * 512], bf16, tag="w",
                                  name="wq_sb")
                nc.sync.dma_start(out=nxt[:], in_=wqP[oc + 1])
                wq_tiles.append(nxt)
            wq_sb = wq_tiles[oc]
            if oc == 2:
                emit_unpack_k()
            if oc == 3:
                emit_unpack_v()
            ssum_q = scr_pool.tile([128, TT * 4], f32, tag="ssum",
                                   name="ssum_q")
            for tt in range(TT):
                ps_q = mm_ps.tile([128, 512], f32, tag="mm", name="ps_q")
                for it in range(IT):
                    nc.tensor.matmul(ps_q[:], hs_tile(it, tt),
                                     wq_sb[:, it * 512:(it + 1) * 512],
                                     start=(it == 0), stop=(it == IT - 1))
                squares(ps_q, 4, ssum_q, tt * 4)
                qst = scr_pool.tile([128, 512], bf16, tag="qps", name="qst",
                                    bufs=4)
                nc.scalar.activation(qst[:], ps_q[:], AF.Copy)
                scq = scales_of(ssum_q[:, tt * 4:(tt + 1) * 4], 4)
                qrope = rope_pool.tile([128, 4 * D], bf16, tag="qrope",
                                       name="qrope", bufs=4)
                rope_apply(qst, 4, scq,
                           cwq_sb[:, tt * D:(tt + 1) * D],
                           swq_sb[:, tt * D:(tt + 1) * D], qrope)
                if oc == 3 or (oc == 2 and tt == 3):
                    xbar_trs.append((oc, tt, qrope))
                else:
                    pending_trs.append(q_tail(oc, tt, qrope))
                boundary()   # one deferred transpose group per tt-chain
                if oc == 0 and tt == 1:
                    emit_gathers()
                    ctx_kv.close()
        while pending_trs:
            boundary()
        # the last 5 transpose groups ride the DMA XBAR on sync: needed
        # only by attention pairs 4-7, ~90us after these land
        for (oc, tt, qrope) in xbar_trs:
            nc.sync.dma_start_transpose(
                out=qT_v[:, oc * 4:(oc + 1) * 4, tt * 128:(tt + 1) * 128],
                in_=qrope[:])
        ctx_hs.close()

        # ================= attention =================
        p2_pool = ep(tc.tile_pool(name="p2", bufs=2))
        attnT_pool = ep(tc.tile_pool(name="attnT", bufs=1))
        den_pool = ep(tc.tile_pool(name="den", bufs=2))
        sc_ps = ep(tc.tile_pool(name="sc_ps", bufs=2, space="PSUM"))
        att_ps = ep(tc.tile_pool(name="att_ps", bufs=4, space="PSUM"))

        attnT_sb = attnT_pool.tile([128, H * SC], bf16)
        # per-pair deferred tail: emitted inside the NEXT pair's st loop so
        # the PE never idles on the DVE denominator chain
        pending_tail = []

        def emit_pair_tail():
            if not pending_tail:
                return
            ha, ps_att, den_bs = pending_tail.pop()
            for hh in range(2):
                h = ha + hh
                ps_db = att_ps.tile([128, SC], f32, tag="att", name="ps_db")
                nc.tensor.matmul(ps_db[:], ones_bf[:], den_bs[hh][:],
                                 start=True, stop=True)
                rden = osb_pool.tile([128, SC], f32, tag="osb", name="rden")
                nc.vector.reciprocal_approx_fast(rden[:], ps_db[:])
                nc.vector.tensor_mul(attnT_sb[:, h * SC:(h + 1) * SC],
                                     ps_att[hh][:], rden[:])

        P2D = 10   # p2 rotation depth: slot st%10.  pair 0 runs with AV
        #            lag 10 (v arrives after the K gather) -- av(st) is
        #            emitted just before exp(st+10) reuses its slot.
        for pr in range(H // 2):
            ha = 2 * pr
            g = ha // (H // KV)
            av_lag = 10 if pr == 0 else 2
            # p2 layout: [128, (slot 10, h 2, n 512)] -> exp writes contiguous
            p2 = p2_pool.tile([128, P2D * 2 * SC], bf16, tag="p2", name="p2")
            p2v = p2.rearrange("p (a h n) -> p h a n", h=2, n=SC)
            ps_att = []     # allocated lazily at st==av_lag (after prev
            accs = [None, None]  # tail's ps_db allocs -- keeps psum slot
                                 # rotation cycle-free)

            def av(st):
                sl = st % P2D
                for hh in range(2):
                    nc.tensor.matmul(
                        ps_att[hh][:], v_tile(g, st),
                        p2[:, sl * 1024 + hh * 512:
                            sl * 1024 + (hh + 1) * 512],
                        start=(st == 0), stop=(st == ST - 1))

            # AV matmuls lag the score matmuls so the PE always has
            # runnable work while exp paces the score psum rotation.
            for st in range(ST):
                sc_t = sc_ps.tile([128, 1024], f32, tag="sc", name="sc_t")
                nc.tensor.matmul(sc_t[:, 0:512], kT_tile(g, st),
                                 qT_sb[:, ha * SC:(ha + 1) * SC],
                                 start=True, stop=True)
                nc.tensor.matmul(sc_t[:, 512:1024], kT_tile(g, st),
                                 qT_sb[:, (ha + 1) * SC:(ha + 2) * SC],
                                 start=True, stop=True)
                if st == 1:
                    emit_pair_tail()   # prev pair's den mm + recip + mul
                if st == av_lag:
                    ps_att.extend(
                        att_ps.tile([128, SC], f32, tag="att", name="ps_att")
                        for _ in range(2))
                if st >= av_lag:
                    # emitted BEFORE exp(st): at lag 10 exp(st) reuses the
                    # very slot av(st-lag) reads
                    av(st - av_lag)
                sl = st % P2D
                nc.scalar.activation(p2[:, sl * 1024:(sl + 1) * 1024],
                                     sc_t[:], AF.Exp)
                # denominator tree-adds spread through the loop (DVE);
                # slot k holds st k for k<10, st 10+k for k<6 afterwards
                if st == 9:
                    for hh in range(2):
                        acc = den_pool.tile([128, 4 * SC], bf16, tag="acc",
                                            name="acc")
                        nc.vector.tensor_add(acc[:], p2v[:, hh, 0:4, :],
                                             p2v[:, hh, 4:8, :])
                        accs[hh] = acc
                if st == 13:
                    # slots 8,9 still hold st8,9; slots 0,1 now hold st10,11
                    for hh in range(2):
                        accv = accs[hh].rearrange("p (a n) -> p a n", n=SC)
                        nc.vector.tensor_add(accv[:, 0:2, :],
                                             accv[:, 0:2, :],
                                             p2v[:, hh, 8:10, :])
                        nc.vector.tensor_add(accv[:, 2:4, :],
                                             accv[:, 2:4, :],
                                             p2v[:, hh, 0:2, :])
            for st in range(ST - av_lag, ST):
                av(st)
            den_bs = []
            for hh in range(2):
                acc = accs[hh]
                accv = acc.rearrange("p (a n) -> p a n", n=SC)
                # slots 2-5 hold st12-15
                nc.vector.tensor_add(accv[:, 0:4, :], accv[:, 0:4, :],
                                     p2v[:, hh, 2:6, :])
                t2b = den_pool.tile([128, 2 * SC], bf16, tag="t2b",
                                    name="t2b")
                nc.vector.tensor_add(t2b[:], acc[:, 0:2 * SC],
                                     acc[:, 2 * SC:4 * SC])
                den_b = den_pool.tile([128, SC], bf16, tag="denb",
                                      name="den_b")
                nc.vector.tensor_add(den_b[:], t2b[:, 0:SC],
                                     t2b[:, SC:2 * SC])
                den_bs.append(den_b)
            pending_tail.append((ha, ps_att, den_bs))

        # ================= output projection =================
        # fl-pairs accumulate in the (now free) score psum slots; the first
        # chain starts while pair 7's softmax tail drains on the DVE.
        for oc in range(OC):
            wo_sb = w_pool.tile([128, IT * 512], bf16, tag="w", name="wo_sb")
            nc.sync.dma_start(out=wo_sb[:], in_=woP[oc])
            for flp in range(2):
                ps_o = sc_ps.tile([128, 1024], f32, tag="sc", name="ps_o")
                for kk in range(IT):
                    for f in range(2):
                        fl = flp * 2 + f
                        nc.tensor.matmul(
                            ps_o[:, f * 512:(f + 1) * 512],
                            wo_sb[:, kk * 512 + fl * 128:
                                  kk * 512 + (fl + 1) * 128],
                            attnT_sb[:, kk * SC:(kk + 1) * SC],
                            start=(kk == 0), stop=(kk == IT - 1))
                    if oc == 0 and flp == 0 and kk == 2:
                        emit_pair_tail()   # pair 7 tail behind these mms
                o_sb = osb_pool.tile([128, 1024], f32, tag="osb2",
                                     name="o_sb")
                nc.vector.tensor_copy(o_sb[:], ps_o[:])
                r0 = (oc * 4 + flp * 2) * 128
                nc.sync.dma_start(
                    out=outT[r0:r0 + 256, :].rearrange(
                        "(f p) n -> p f n", p=128),
                    in_=o_sb.rearrange("p (f n) -> p f n", f=2))
    finally:
        ctx.close()
        tc_cm.__exit__(None, None, None)

    nc.compile()
    return nc


def _prep_inputs(hidden_states, cos, sin, Wq, Wk, Wv, Wo, norm_q_w,
                 norm_k_w):
    """Host-side: transpose + bf16-cast weights into tile-major layouts,
    fold norm weights + 1/sqrt(D) into rope coefficients, slice per core."""
    import ml_dtypes
    f = np.float32
    bf = ml_dtypes.bfloat16
    hs = np.asarray(hidden_states, f)
    cos = np.asarray(cos, f)
    sin = np.asarray(sin, f)

    def tile_major(wT, oc_split):
        # wT: [HS, N] -> [oc][128, it, 512] (tile-major over rows)
        n = wT.shape[1]
        arr = wT.reshape(IT, 128, n)
        if oc_split:
            out = np.empty((OC, 128, IT * 512), bf)
            for oc in range(OC):
                blk = arr[:, :, oc * 512:(oc + 1) * 512]  # [it, 128, 512]
                out[oc] = blk.transpose(1, 0, 2).reshape(128, IT * 512)
            return out
        return np.ascontiguousarray(
            arr.transpose(1, 0, 2).reshape(128, IT * 512)).astype(bf)

    wq = tile_major(np.asarray(Wq, f).T, True)       # [4, 128, 8192]
    wk = tile_major(np.asarray(Wk, f).T, False)      # [128, 8192]
    wv = tile_major(np.asarray(Wv, f).T, False)
    wo = tile_major(np.asarray(Wo, f).T, True)
    wqn = np.asarray(norm_q_w, f)
    wkn = np.asarray(norm_k_w, f)

    def rope_consts(w, scale):
        # cw[t, d] = cos[t, d] * w[d] * scale
        # sw[t, d<64]  = -sin[t, d] * w[d+64] * scale
        # sw[t, d>=64] = +sin[t, d] * w[d-64] * scale
        cw = cos * w[None, :] * scale
        w_swap = np.concatenate([w[D // 2:], w[:D // 2]])
        sgn = np.concatenate([-np.ones(D // 2, f), np.ones(D // 2, f)])
        sw = sin * (w_swap * sgn)[None, :] * scale
        return cw.astype(f), sw.astype(f)

    cwq_full, swq_full = rope_consts(wqn, np.float32(D ** -0.5))
    cwk_full, swk_full = rope_consts(wkn, np.float32(1.0))

    def part_major(a):
        # [512, D] -> [128, tt, D] -> [128, tt*D]
        return np.ascontiguousarray(
            a.reshape(TT, 128, D).transpose(1, 0, 2).reshape(128, TT * D))

    in_maps = []
    for c in range(NCORES):
        b, j = divmod(c, TPG)
        sl = slice(j * SC, (j + 1) * SC)
        hsT = hs[b].T[:, sl]                          # [2048 i, 512 t]
        hsp = hsT.reshape(IT, 128, SC).transpose(1, 0, 2).reshape(
            128, IT * SC).astype(bf)
        in_maps.append({
            "hsP": np.ascontiguousarray(hsp),
            "cwq": part_major(cwq_full[sl]),
            "swq": part_major(swq_full[sl]),
            "cwk": part_major(cwk_full[sl]),
            "swk": part_major(swk_full[sl]),
            "wqP": wq, "wkP": wk, "wvP": wv, "woP": wo,
        })
    return in_maps


def _assemble(results):
    out = np.empty((B, S, HS), np.float32)
    for c in range(NCORES):
        b, j = divmod(c, TPG)
        out[b, j * SC:(j + 1) * SC, :] = results[c]["outT"].T
    return out


def kernel(hidden_states, cos, sin, Wq, Wk, Wv, Wo, norm_q_w, norm_k_w,
           _run_kwargs=None):
    from concourse.bass_utils import run_bass_kernel_spmd

    if "nc" not in _BUILT:
        _BUILT["nc"] = _build_program()
    nc = _BUILT["nc"]
    in_maps = _prep_inputs(hidden_states, cos, sin, Wq, Wk, Wv, Wo,
                           norm_q_w, norm_k_w)
    kw = _run_kwargs or {}
    res = run_bass_kernel_spmd(nc, in_maps, list(range(NCORES)), **kw)
    _BUILT["last_results"] = res
    return _assemble(res.results)


# revision 35
# speedup vs baseline: 1.0884x; 1.0884x over previous
"""AceStep GQA attention block on 8 Trainium2 NeuronCores.

Sharding: DP over batch (B=2) x sequence-parallel over S within each batch
group (4 cores each own 512 query positions).  Each core computes K/V for its
own 512 positions, AllGathers K^T/V (bf16) within its 4-core group, then runs
full attention for its query slice and the complete output projection row
block.  No output collective needed - each core owns a distinct slice.

v7 schedule (vs v2 at ~409us measured): the PE issues N=512 bf16 matmuls at a
steady ~263ns (power-throttle ceiling); all remaining headroom is PE idle
gaps, so v7 is a pure scheduling restructure:
  - input DMAs chunked and split across the two HWDGE queues (sync: weights
    + packs/unpacks, scalar: hidden/rope consts); w_pool is 3-deep so each
    wq chunk prefetches a full block ahead.  gpsimd carries ONLY the
    collective triggers (software DMAs there contend with the CC DSP).
  - K projection runs FIRST: measured, the first collective cannot finish
    before ~90us regardless of issue time (ring init + cross-core skew), so
    the K gather -- whose payload attention needs first -- is the one that
    absorbs that floor.  V follows (its gather rides second; attention's
    AV matmuls lag scores anyway, and pair 0 uses a deeper AV lag to ride
    out the later v arrival).  Q projection overlaps both transfers.
  - per-tile norm scales (Copy+reciprocal+Sqrt -- one ACT table set with
    Square, no Ln/Exp thrash) + an ACT psum->SBUF stage right after each
    chain's squares: psum banks recycle ~0.7us after their chain and the
    DVE ropes pipeline tt-by-tt under the matmuls.
  - K/Q rope->transpose tails run on the PE but are DEFERRED two tt-chains
    past their rope's ready time (in-order PE queue): each 4-transpose
    group is emitted at a later chain boundary where its DVE rope is
    provably done, so the PE never stalls.  The last 5 groups (oc2-tt3,
    oc3) go through the DMA XBAR on sync -- needed only by attention pairs
    4-7, ~90us after they land.
  - softmax denominator tree-adds interleave into the score loop (p2 is a
    10-slot rotating buffer); each pair's den-matmul/reciprocal/attnT tail
    is emitted inside the NEXT pair's loop so the PE never waits on DVE.
  - O projection accumulates fl-pairs in the score psum slots ([128,1024]
    double-buffered, freed by then) and starts during the last pair's
    softmax tail; output DMAs are 8x512KB contiguous on sync

Device layouts (per core, c = 4*b + j):
  hsP   [128, it 16, t 512]  hidden[b].T tile-major (partition = i % 128)
  wkP/wvP [128, it 16, n 512]   W^T tile-major
  wqP/woP [oc 4][128, it/kk 16, n 512]
  cw*/sw* [128, tt 4, d 128]  rope coeffs (norm weight + scale folded in)
  outT  [HS, SC] f32
"""

import numpy as np

H, KV, D = 16, 4, 128
HD = D // 2
B, S, HS = 2, 2048, 2048
EPS = 1e-6
NCORES = 8
TPG = 4              # cores per batch group (sequence split)
SC = S // TPG        # 512 sequence positions per core
TT = SC // 128       # 4 t-tiles per core
IT = HS // 128       # 16 contraction tiles
ST = S // 128        # 16 s-tiles (full sequence)
OC = 4               # 512-wide output chunks for Q/O projections
GROUPS = [[0, 1, 2, 3], [4, 5, 6, 7]]

_BUILT = {}


def _build_program():
    from contextlib import ExitStack

    import concourse.bass as bass
    import concourse.bacc as bacc
    import concourse.mybir as mybir
    import concourse.tile as tile
    from concourse.masks import make_identity

    f32 = mybir.dt.float32
    bf16 = mybir.dt.bfloat16
    AF = mybir.ActivationFunctionType
    ALU = mybir.AluOpType

    nc = bacc.Bacc("TRN2", target_bir_lowering=False, debug=False,
                   num_devices=NCORES)

    # ---- external I/O (per core) ----
    hsP = nc.dram_tensor("hsP", [128, IT * SC], bf16, kind="ExternalInput").ap()
    wkP = nc.dram_tensor("wkP", [128, IT * 512], bf16, kind="ExternalInput").ap()
    wvP = nc.dram_tensor("wvP", [128, IT * 512], bf16, kind="ExternalInput").ap()
    wqP = nc.dram_tensor("wqP", [OC, 128, IT * 512], bf16,
                         kind="ExternalInput").ap()
    woP = nc.dram_tensor("woP", [OC, 128, IT * 512], bf16,
                         kind="ExternalInput").ap()
    cwq = nc.dram_tensor("cwq", [128, TT * D], f32, kind="ExternalInput").ap()
    swq = nc.dram_tensor("swq", [128, TT * D], f32, kind="ExternalInput").ap()
    cwk = nc.dram_tensor("cwk", [128, TT * D], f32, kind="ExternalInput").ap()
    swk = nc.dram_tensor("swk", [128, TT * D], f32, kind="ExternalInput").ap()
    outT = nc.dram_tensor("outT", [HS, SC], f32, kind="ExternalOutput").ap()

    tc_cm = tile.TileContext(nc)
    ctx = ExitStack()
    tc = tc_cm.__enter__()
    try:
        ep = ctx.enter_context
        const_pool = ep(tc.tile_pool(name="const", bufs=1))
        w_pool = ep(tc.tile_pool(name="w", bufs=3))
        scr_pool = ep(tc.tile_pool(name="scr", bufs=2))
        rope_pool = ep(tc.tile_pool(name="rope", bufs=4))
        qT_pool = ep(tc.tile_pool(name="qT", bufs=1))
        kv_pool = ep(tc.tile_pool(name="kv", bufs=1))
        osb_pool = ep(tc.tile_pool(name="osb", bufs=2))
        dram_pool = ep(tc.tile_pool(name="dram", bufs=1, space="DRAM"))

        # scoped pools released before the attention phase needs the space
        # (pool releases must be LIFO: kvloc closes first, then hs)
        ctx_hs = ExitStack()      # hs + projection psum (until Q proj done)
        ctx_kv = ExitStack()      # local kT/v staging (until pack DMAs done)
        hs_pool = ctx_hs.enter_context(tc.tile_pool(name="hs", bufs=1))
        mm_ps = ctx_hs.enter_context(
            tc.tile_pool(name="mm_ps", bufs=6, space="PSUM"))
        tr_ps = ctx_hs.enter_context(
            tc.tile_pool(name="tr_ps", bufs=2, space="PSUM"))
        kvloc_pool = ctx_kv.enter_context(tc.tile_pool(name="kvloc", bufs=1))

        # ---- constants ----
        ident = const_pool.tile([128, 128], bf16)
        make_identity(nc, ident)
        ones_bf = const_pool.tile([128, 128], bf16)
        nc.vector.memset(ones_bf, 1.0)
        eps_sb = const_pool.tile([128, 1], f32)
        nc.vector.memset(eps_sb, EPS)

        # hidden states: 4 chunks on the scalar (ACT) HWDGE queue so the
        # first matmul can start ~11us in; weights stream on sync
        hs_sb = hs_pool.tile([128, IT * SC], bf16)
        # first chunk small so the first K chain starts ASAP
        hcuts = [0, 512, 2048, 4096, 6144, IT * SC]
        for a, b in zip(hcuts, hcuts[1:]):
            nc.scalar.dma_start(out=hs_sb[:, a:b], in_=hsP[:, a:b])

        cwq_sb = const_pool.tile([128, TT * D], f32)
        swq_sb = const_pool.tile([128, TT * D], f32)
        cwk_sb = const_pool.tile([128, TT * D], f32)
        swk_sb = const_pool.tile([128, TT * D], f32)
        for (dst, src) in ((cwk_sb, cwk), (swk_sb, swk),
                           (cwq_sb, cwq), (swq_sb, swq)):
            nc.scalar.dma_start(out=dst[:], in_=src)

        def hs_tile(it, tt):
            # stationary [128 i, 128 t]
            off = it * SC + tt * 128
            return hs_sb[:, off: off + 128]

        def squares(ps, nh, ssum, base):
            """sum(x^2) over D per head via ACT Square + accum_out."""
            for hh in range(nh):
                sqd = scr_pool.tile([128, D], f32, tag="sqd", name="sqd")
                nc.scalar.activation(
                    sqd[:], ps[:, hh * D:(hh + 1) * D], AF.Square,
                    accum_out=ssum[:, base + hh:base + hh + 1])

        def scales_of(ssum_sl, n):
            """rsqrt(x/D + eps) = sqrt(1/(x/D + eps)): Copy + Sqrt share one
            ACT table set with Square -> the projections never swap tables
            (Exp loads once for attention)."""
            mn = scr_pool.tile([128, n], f32, tag="lnm", name="mn")
            nc.scalar.activation(mn[:], ssum_sl, AF.Copy, bias=EPS,
                                 scale=1.0 / D)
            rp = scr_pool.tile([128, n], f32, tag="rp", name="rp")
            nc.vector.reciprocal(rp[:], mn[:])
            sc_t = scr_pool.tile([128, n], f32, tag="sct", name="sc_t")
            nc.scalar.activation(sc_t[:], rp[:], AF.Sqrt)
            return sc_t

        def rope_apply(xsb, nh, sc_sl, cw_t, sw_t, dst):
            """xsb: SBUF bf16 [128 t, nh*D]; sc_sl: [128, nh] scales;
            dst: bf16 [128, nh*D].  All heads batched per DVE op."""
            scf = scr_pool.tile([128, nh * D], bf16, tag="scf", name="scf")
            nc.vector.tensor_copy(
                scf.rearrange("p (h d) -> p h d", d=D),
                sc_sl.rearrange("p (h one) -> p h one", one=1).broadcast_to(
                    [128, nh, D]))
            xs = scr_pool.tile([128, nh * D], bf16, tag="xs", name="xs")
            nc.vector.tensor_mul(xs[:], xsb[:], scf[:])
            t1 = scr_pool.tile([128, nh * D], bf16, tag="t1", name="t1")
            cwb = cw_t.rearrange("p (one d) -> p one d", one=1).broadcast_to(
                [128, nh, D])
            nc.vector.tensor_mul(t1.rearrange("p (h d) -> p h d", d=D),
                                 xs.rearrange("p (h d) -> p h d", d=D), cwb)
            t2 = scr_pool.tile([128, nh * D], bf16, tag="t2", name="t2")
            xsv = xs.rearrange("p (h two x) -> p h two x", two=2, x=HD)
            t2v = t2.rearrange("p (h two x) -> p h two x", two=2, x=HD)
            swb_lo = sw_t[:, 0:HD].rearrange(
                "p (one x) -> p one x", one=1).broadcast_to([128, nh, HD])
            swb_hi = sw_t[:, HD:D].rearrange(
                "p (one x) -> p one x", one=1).broadcast_to([128, nh, HD])
            nc.vector.tensor_mul(t2v[:, :, 0, :], xsv[:, :, 1, :], swb_lo)
            nc.vector.tensor_mul(t2v[:, :, 1, :], xsv[:, :, 0, :], swb_hi)
            nc.vector.tensor_add(dst[:], t1[:], t2[:])

        # deferred PE-transpose groups, popped one per tt-chain boundary
        # (shift-2: two leading Nones) so the in-order PE queue never
        # stalls on a DVE rope.
        pending_trs = [None, None]

        def boundary():
            if pending_trs:
                emit = pending_trs.pop(0)
                if emit is not None:
                    emit()

        # ================= K projection (first) =================
        wk_sb = w_pool.tile([128, IT * 512], bf16, tag="w", name="wk_sb")
        wcuts = [0, 512, 2048, 4096, 6144, IT * 512]
        for a, b in zip(wcuts, wcuts[1:]):
            nc.sync.dma_start(out=wk_sb[:, a:b], in_=wkP[:, a:b])
        wv_sb = w_pool.tile([128, IT * 512], bf16, tag="w", name="wv_sb")
        NWC = 4
        wchunk = IT * 512 // NWC
        for c in range(NWC):
            nc.sync.dma_start(out=wv_sb[:, c * wchunk:(c + 1) * wchunk],
                              in_=wvP[:, c * wchunk:(c + 1) * wchunk])
        wq_sb0 = w_pool.tile([128, IT * 512], bf16, tag="w", name="wq_sb")
        nc.sync.dma_start(out=wq_sb0[:], in_=wqP[0])

        kTs_sb = kvloc_pool.tile([128, KV * SC], bf16)  # [d, (g s)] local kT
        v_sb = kvloc_pool.tile([128, TT * KV * D], bf16)  # [s,(st hd)] local
        kTs_v = kTs_sb.rearrange("p (g s) -> p g s", s=SC)

        def k_tail(st, krope):
            def emit():
                pst = tr_ps.tile([128, KV * 128], bf16, tag="tr", name="pst")
                for g in range(KV):
                    nc.tensor.transpose(pst[:, g * 128:(g + 1) * 128],
                                        krope[:, g * D:(g + 1) * D],
                                        ident[:])
                nc.vector.tensor_copy(
                    kTs_v[:, :, st * 128:(st + 1) * 128],
                    pst.rearrange("p (g t) -> p g t", t=128))
            return emit

        ssum_k = scr_pool.tile([128, TT * KV], f32, tag="ssum", name="ssum_k")
        for st in range(TT):
            ps_k = mm_ps.tile([128, KV * D], f32, tag="mm", name="ps_k")
            for it in range(IT):
                nc.tensor.matmul(ps_k[:], hs_tile(it, st),
                                 wk_sb[:, it * 512:(it + 1) * 512],
                                 start=(it == 0), stop=(it == IT - 1))
            squares(ps_k, KV, ssum_k, st * KV)
            kst = scr_pool.tile([128, KV * D], bf16, tag="qps", name="kst",
                                bufs=4)
            nc.scalar.activation(kst[:], ps_k[:], AF.Copy)
            sck = scales_of(ssum_k[:, st * KV:(st + 1) * KV], KV)
            krope = rope_pool.tile([128, KV * D], bf16, tag="krope",
                                   name="krope")
            rope_apply(kst, KV, sck,
                       cwk_sb[:, st * D:(st + 1) * D],
                       swk_sb[:, st * D:(st + 1) * D], krope)
            pending_trs.append(k_tail(st, krope))

        # ================= V projection =================
        for st in range(TT):
            ps_v = mm_ps.tile([128, KV * D], f32, tag="mm", name="ps_v")
            for it in range(IT):
                nc.tensor.matmul(ps_v[:], hs_tile(it, st),
                                 wv_sb[:, it * 512:(it + 1) * 512],
                                 start=(it == 0), stop=(it == IT - 1))
            nc.vector.tensor_copy(v_sb[:, st * KV * D:(st + 1) * KV * D],
                                  ps_v[:])
            boundary()

        # v pack rides sync early; the CC triggers (gpsimd) are emitted
        # k-first at the oc0-tt1 boundary below.  Payloads are
        # partition-major [128, 2048] so every pack/unpack moves 4KB rows
        # (full-bandwidth descriptors, no rearrange).
        cc_in_k = dram_pool.tile([128, KV * SC], bf16)
        cc_out_k = dram_pool.tile([TPG * 128, KV * SC], bf16)
        cc_in_v = dram_pool.tile([128, TT * KV * D], bf16)
        cc_out_v = dram_pool.tile([TPG * 128, TT * KV * D], bf16)
        nc.sync.dma_start(out=cc_in_v[:], in_=v_sb[:])

        def emit_gathers():
            nc.sync.dma_start(out=cc_in_k[:], in_=kTs_sb[:])
            nc.gpsimd.collective_compute(
                "AllGather", ALU.bypass, replica_groups=GROUPS,
                ins=[cc_in_k.opt()], outs=[cc_out_k.opt()])
            nc.gpsimd.collective_compute(
                "AllGather", ALU.bypass, replica_groups=GROUPS,
                ins=[cc_in_v.opt()], outs=[cc_out_v.opt()])

        # gathered K/V landing buffers; unpacks emitted inside the Q loop
        kT_big = kv_pool.tile([128, KV * S], bf16, tag="kT")    # [d,(rr g s)]
        v_big = kv_pool.tile([128, ST * KV * D], bf16, tag="v")  # [s,(st hd)]

        def emit_unpack_v():
            # gpsimd: keeps the late-arriving v unpack's semaphores out of
            # the shared sync/scalar HWDGE pool (a recycle wait there gated
            # the attention exps on CC-v completion)
            for rr in range(TPG):
                nc.gpsimd.dma_start(
                    out=v_big[:, rr * TT * KV * D:(rr + 1) * TT * KV * D],
                    in_=cc_out_v[rr * 128:(rr + 1) * 128, :])

        def emit_unpack_k():
            for rr in range(TPG):
                nc.sync.dma_start(
                    out=kT_big[:, rr * KV * SC:(rr + 1) * KV * SC],
                    in_=cc_out_k[rr * 128:(rr + 1) * 128, :])

        def kT_tile(g, st):
            # stationary [128 d, 128 s] for s-tile st (st = rr*4 + sub)
            rr, sub = st // TT, st % TT
            off = (rr * KV + g) * SC + sub * 128
            return kT_big[:, off: off + 128]

        def v_tile(g, st):
            # stationary [128 s, 128 d] for s-tile st
            off = st * KV * D + g * D
            return v_big[:, off: off + D]

        # ================= Q projection (overlaps collectives) ===========
        qT_sb = qT_pool.tile([128, H * SC], bf16)   # per head: [d, 512 t]
        qT_v = qT_sb.rearrange("p (h t) -> p h t", t=SC)

        def q_tail(oc, tt, qrope):
            def emit():
                pstq = tr_ps.tile([128, 4 * 128], bf16, tag="tr", name="pstq")
                for hh in range(4):
                    nc.tensor.transpose(pstq[:, hh * 128:(hh + 1) * 128],
                                        qrope[:, hh * D:(hh + 1) * D],
                                        ident[:])
                nc.vector.tensor_copy(
                    qT_v[:, oc * 4:(oc + 1) * 4, tt * 128:(tt + 1) * 128],
                    pstq.rearrange("p (h t) -> p h t", t=128))
            return emit

        xbar_trs = []   # (oc, tt, qrope) for the last 5 groups -> DMA XBAR
        wq_tiles = [wq_sb0]

        for oc in range(OC):
            if oc + 1 < OC:
                # prefetch the next oc's weights one block ahead
                nxt = w_pool.tile([128, IT * 512], bf16, tag="w",
                                  name="wq_sb")
                nc.sync.dma_start(out=nxt[:], in_=wqP[oc + 1])
                wq_tiles.append(nxt)
            wq_sb = wq_tiles[oc]
            if oc == 2:
                emit_unpack_k()
            if oc == 3:
                emit_unpack_v()
            ssum_q = scr_pool.tile([128, TT * 4], f32, tag="ssum",
                                   name="ssum_q")
            for tt in range(TT):
                ps_q = mm_ps.tile([128, 512], f32, tag="mm", name="ps_q")
                for it in range(IT):
                    nc.tensor.matmul(ps_q[:], hs_tile(it, tt),
                                     wq_sb[:, it * 512:(it + 1) * 512],
                                     start=(it == 0), stop=(it == IT - 1))
                squares(ps_q, 4, ssum_q, tt * 4)
                qst = scr_pool.tile([128, 512], bf16, tag="qps", name="qst",
                                    bufs=4)
                nc.scalar.activation(qst[:], ps_q[:], AF.Copy)
                scq = scales_of(ssum_q[:, tt * 4:(tt + 1) * 4], 4)
                qrope = rope_pool.tile([128, 4 * D], bf16, tag="qrope",
                                       name="qrope", bufs=4)
                rope_apply(qst, 4, scq,
                           cwq_sb[:, tt * D:(tt + 1) * D],
                           swq_sb[:, tt * D:(tt + 1) * D], qrope)
                if oc == 3 or (oc == 2 and tt == 3):
                    xbar_trs.append((oc, tt, qrope))
                else:
                    pending_trs.append(q_tail(oc, tt, qrope))
                boundary()   # one deferred transpose group per tt-chain
                if oc == 0 and tt == 1:
                    emit_gathers()
                    ctx_kv.close()
        while pending_trs:
            boundary()
        # the last 5 transpose groups ride the DMA XBAR on sync: needed
        # only by attention pairs 4-7, ~90us after these land
        for (oc, tt, qrope) in xbar_trs:
            nc.sync.dma_start_transpose(
                out=qT_v[:, oc * 4:(oc + 1) * 4, tt * 128:(tt + 1) * 128],
                in_=qrope[:])
        ctx_hs.close()

        # ================= attention =================
        p2_pool = ep(tc.tile_pool(name="p2", bufs=2))
        attnT_pool = ep(tc.tile_pool(name="attnT", bufs=1))
        den_pool = ep(tc.tile_pool(name="den", bufs=2))
        sc_ps = ep(tc.tile_pool(name="sc_ps", bufs=2, space="PSUM"))
        att_ps = ep(tc.tile_pool(name="att_ps", bufs=4, space="PSUM"))

        attnT_sb = attnT_pool.tile([128, H * SC], bf16)
        # per-pair deferred tail: emitted inside the NEXT pair's st loop so
        # the PE never idles on the DVE denominator chain
        pending_tail = []

        def emit_pair_tail():
            if not pending_tail:
                return
            ha, ps_att, den_bs = pending_tail.pop()
            for hh in range(2):
                h = ha + hh
                ps_db = att_ps.tile([128, SC], f32, tag="att", name="ps_db")
                nc.tensor.matmul(ps_db[:], ones_bf[:], den_bs[hh][:],
                                 start=True, stop=True)
                rden = osb_pool.tile([128, SC], f32, tag="osb", name="rden")
                nc.vector.reciprocal_approx_fast(rden[:], ps_db[:])
                nc.vector.tensor_mul(attnT_sb[:, h * SC:(h + 1) * SC],
                                     ps_att[hh][:], rden[:])

        P2D = 10   # p2 rotation depth: slot st%10.  pair 0 runs with AV
        #            lag 10 (v arrives after the K gather) -- av(st) is
        #            emitted just before exp(st+10) reuses its slot.
        for pr in range(H // 2):
            ha = 2 * pr
            g = ha // (H // KV)
            av_lag = 10 if pr == 0 else 2
            # p2 layout: [128, (slot 10, h 2, n 512)] -> exp writes contiguous
            p2 = p2_pool.tile([128, P2D * 2 * SC], bf16, tag="p2", name="p2")
            p2v = p2.rearrange("p (a h n) -> p h a n", h=2, n=SC)
            ps_att = []     # allocated lazily at st==av_lag (after prev
            accs = [None, None]  # tail's ps_db allocs -- keeps psum slot
                                 # rotation cycle-free)

            def av(st):
                sl = st % P2D
                for hh in range(2):
                    nc.tensor.matmul(
                        ps_att[hh][:], v_tile(g, st),
                        p2[:, sl * 1024 + hh * 512:
                            sl * 1024 + (hh + 1) * 512],
                        start=(st == 0), stop=(st == ST - 1))

            # AV matmuls lag the score matmuls so the PE always has
            # runnable work while exp paces the score psum rotation.
            for st in range(ST):
                sc_t = sc_ps.tile([128, 1024], f32, tag="sc", name="sc_t")
                nc.tensor.matmul(sc_t[:, 0:512], kT_tile(g, st),
                                 qT_sb[:, ha * SC:(ha + 1) * SC],
                                 start=True, stop=True)
                nc.tensor.matmul(sc_t[:, 512:1024], kT_tile(g, st),
                                 qT_sb[:, (ha + 1) * SC:(ha + 2) * SC],
                                 start=True, stop=True)
                if st == 1:
                    emit_pair_tail()   # prev pair's den mm + recip + mul
                if st == av_lag:
                    ps_att.extend(
                        att_ps.tile([128, SC], f32, tag="att", name="ps_att")
                        for _ in range(2))
                if st >= av_lag:
                    # emitted BEFORE exp(st): at lag 10 exp(st) reuses the
                    # very slot av(st-lag) reads
                    av(st - av_lag)
                sl = st % P2D
                nc.scalar.activation(p2[:, sl * 1024:(sl + 1) * 1024],
                                     sc_t[:], AF.Exp)
                # denominator tree-adds spread through the loop (DVE);
                # slot k holds st k for k<10, st 10+k for k<6 afterwards
                if st == 9:
                    for hh in range(2):
                        acc = den_pool.tile([128, 4 * SC], bf16, tag="acc",
                                            name="acc")
                        nc.vector.tensor_add(acc[:], p2v[:, hh, 0:4, :],
                                             p2v[:, hh, 4:8, :])
                        accs[hh] = acc
                if st == 13:
                    # slots 8,9 still hold st8,9; slots 0,1 now hold st10,11
                    for hh in range(2):
                        accv = accs[hh].rearrange("p (a n) -> p a n", n=SC)
                        nc.vector.tensor_add(accv[:, 0:2, :],
                                             accv[:, 0:2, :],
                                             p2v[:, hh, 8:10, :])
                        nc.vector.tensor_add(accv[:, 2:4, :],
                                             accv[:, 2:4, :],
                                             p2v[:, hh, 0:2, :])
            for st in range(ST - av_lag, ST):
                av(st)
            den_bs = []
            for hh in range(2):
                acc = accs[hh]
                accv = acc.rearrange("p (a n) -> p a n", n=SC)
                # slots 2-5 hold st12-15
                nc.vector.tensor_add(accv[:, 0:4, :], accv[:, 0:4, :],
                                     p2v[:, hh, 2:6, :])
                t2b = den_pool.tile([128, 2 * SC], bf16, tag="t2b",
                                    name="t2b")
                nc.vector.tensor_add(t2b[:], acc[:, 0:2 * SC],
                                     acc[:, 2 * SC:4 * SC])
                den_b = den_pool.tile([128, SC], bf16, tag="denb",
                                      name="den_b")
                nc.vector.tensor_add(den_b[:], t2b[:, 0:SC],
                                     t2b[:, SC:2 * SC])
                den_bs.append(den_b)
            pending_tail.append((ha, ps_att, den_bs))

        # ================= output projection =================
        # fl-pairs accumulate in the (now free) score psum slots; the first
        # chain starts while pair 7's softmax tail drains on the DVE.
        for oc in range(OC):
            wo_sb = w_pool.tile([128, IT * 512], bf16, tag="w", name="wo_sb")
            nc.sync.dma_start(out=wo_sb[:], in_=woP[oc])
            for flp in range(2):
                ps_o = sc_ps.tile([128, 1024], f32, tag="sc", name="ps_o")
                for kk in range(IT):
                    for f in range(2):
                        fl = flp * 2 + f
                        nc.tensor.matmul(
                            ps_o[:, f * 512:(f + 1) * 512],
                            wo_sb[:, kk * 512 + fl * 128:
                                  kk * 512 + (fl + 1) * 128],
                            attnT_sb[:, kk * SC:(kk + 1) * SC],
                            start=(kk == 0), stop=(kk == IT - 1))
                    if oc == 0 and flp == 0 and kk == 2:
                        emit_pair_tail()   # pair 7 tail behind these mms
                o_sb = osb_pool.tile([128, 1024], f32, tag="osb2",
                                     name="o_sb")
                nc.vector.tensor_copy(o_sb[:], ps_o[:])
                r0 = (oc * 4 + flp * 2) * 128
                nc.sync.dma_start(
                    out=outT[r0:r0 + 256, :].rearrange(
                        "(f p) n -> p f n", p=128),
                    in_=o_sb.rearrange("p (f n) -> p f n", f=2))
    finally:
        ctx.close()
        tc_cm.__exit__(None, None, None)

    nc.compile()
    return nc


def _prep_inputs(hidden_states, cos, sin, Wq, Wk, Wv, Wo, norm_q_w,
                 norm_k_w):
    """Host-side: transpose + bf16-cast weights into tile-major layouts,
    fold norm weights + 1/sqrt(D) into rope coefficients, slice per core."""
    import ml_dtypes
    f = np.float32
    bf = ml_dtypes.bfloat16
    hs = np.asarray(hidden_states, f)
    cos = np.asarray(cos, f)
    sin = np.asarray(sin, f)

    def tile_major(wT, oc_split):
        # wT: [HS, N] -> [oc][128, it, 512] (tile-major over rows)
        n = wT.shape[1]
        arr = wT.reshape(IT, 128, n)
        if oc_split:
            out = np.empty((OC, 128, IT * 512), bf)
            for oc in range(OC):
                blk = arr[:, :, oc * 512:(oc + 1) * 512]  # [it, 128, 512]
                out[oc] = blk.transpose(1, 0, 2).reshape(128, IT * 512)
            return out
        return np.ascontiguousarray(
            arr.transpose(1, 0, 2).reshape(128, IT * 512)).astype(bf)

    wq = tile_major(np.asarray(Wq, f).T, True)       # [4, 128, 8192]
    wk = tile_major(np.asarray(Wk, f).T, False)      # [128, 8192]
    wv = tile_major(np.asarray(Wv, f).T, False)
    wo = tile_major(np.asarray(Wo, f).T, True)
    wqn = np.asarray(norm_q_w, f)
    wkn = np.asarray(norm_k_w, f)

    def rope_consts(w, scale):
        # cw[t, d] = cos[t, d] * w[d] * scale
        # sw[t, d<64]  = -sin[t, d] * w[d+64] * scale
        # sw[t, d>=64] = +sin[t, d] * w[d-64] * scale
        cw = cos * w[None, :] * scale
        w_swap = np.concatenate([w[D // 2:], w[:D // 2]])
        sgn = np.concatenate([-np.ones(D // 2, f), np.ones(D // 2, f)])
        sw = sin * (w_swap * sgn)[None, :] * scale
        return cw.astype(f), sw.astype(f)

    cwq_full, swq_full = rope_consts(wqn, np.float32(D ** -0.5))
    cwk_full, swk_full = rope_consts(wkn, np.float32(1.0))

    def part_major(a):
        # [512, D] -> [128, tt, D] -> [128, tt*D]
        return np.ascontiguousarray(
            a.reshape(TT, 128, D).transpose(1, 0, 2).reshape(128, TT * D))

    in_maps = []
    for c in range(NCORES):
        b, j = divmod(c, TPG)
        sl = slice(j * SC, (j + 1) * SC)
        hsT = hs[b].T[:, sl]                          # [2048 i, 512 t]
        hsp = hsT.reshape(IT, 128, SC).transpose(1, 0, 2).reshape(
            128, IT * SC).astype(bf)
        in_maps.append({
            "hsP": np.ascontiguousarray(hsp),
            "cwq": part_major(cwq_full[sl]),
            "swq": part_major(swq_full[sl]),
            "cwk": part_major(cwk_full[sl]),
            "swk": part_major(swk_full[sl]),
            "wqP": wq, "wkP": wk, "wvP": wv, "woP": wo,
        })
    return in_maps


def _assemble(results):
    out = np.empty((B, S, HS), np.float32)
    for c in range(NCORES):
        b, j = divmod(c, TPG)
        out[b, j * SC:(j + 1) * SC, :] = results[c]["outT"].T
    return out


def kernel(hidden_states, cos, sin, Wq, Wk, Wv, Wo, norm_q_w, norm_k_w,
           _run_kwargs=None):
    from concourse.bass_utils import run_bass_kernel_spmd

    if "nc" not in _BUILT:
        _BUILT["nc"] = _build_program()
    nc = _BUILT["nc"]
    in_maps = _prep_inputs(hidden_states, cos, sin, Wq, Wk, Wv, Wo,
                           norm_q_w, norm_k_w)
    kw = _run_kwargs or {}
    res = run_bass_kernel_spmd(nc, in_maps, list(range(NCORES)), **kw)
    _BUILT["last_results"] = res
    return _assemble(res.results)
